# revision 2
# baseline (speedup 1.0000x reference)
"""Trainium2 Bass kernel for nn_ConvPair (pairwise-MLP message passing).

Reference computation (N=1024 atoms, F=8 feats, H=128 hidden, O=3 out):
    hi = x @ W1[:F];  hj = x @ W1[F:]
    h  = tanh(hi[:,None,:] + hj[None,:,:] + b1)        # [N,N,H]
    h  = tanh(h @ W2 + b2)                             # [N,N,H]
    y  = tanh(h @ W3 + b3)                             # [N,N,O]
    out = y.sum(axis=(1,2))                            # [N]

Sharding: outer atom dim i split across 8 cores (128 i per core); the small
weights and the full atom table are replicated. No cross-core reduction.

Per-core device pipeline, all tensors hidden-major [H=128 partitions, ...]:
  tanh1: ACT  tanh(HJ_T + hib_i)        one instr per i, bias = HIB col
  mm1:   PE   W2.T-contract             2 matmuls (N=512 each) -> PSUM
  tanh2: ACT  tanh(psum + b2) -> SBUF   bias = b2 column
  mm3:   PE   8x (h2_chunk.T @ W3pad)   pairs-on-partitions -> PSUM [128,32]
  tanh3: ACT  tanh(psum), accum_out     per-channel accumulator -> ACC[:,i]
  final: PE   ACC.T @ ones  ->  per-i scalars, ACT copy, DMA out.

b1 is folded into hib host-side; b2 via the ACT bias port; b3 is zeros for
this problem (asserted; a numpy fallback handles the hypothetical nonzero
case exactly).

Wait-discipline: walrus's Activation codegen supports only one semaphore
wait per instruction, so all constants arrive in ONE DMA and each engine
"touches" that DMA's semaphore once in a warmup instruction; afterwards the
steady-state loop only ever ping-pongs ACT<->PE (one foreign sem each).
"""

import json

import numpy as np
from contextlib import ExitStack

import bass_rust
import concourse.bass as bass
import concourse.tile as tile
from concourse import mybir
from concourse.bass_utils import run_bass_kernel_spmd

f32 = mybir.dt.float32
Tanh = mybir.ActivationFunctionType.Tanh

N, F, H, O = 1024, 8, 128, 3
NCORES = 8
IPC = N // NCORES  # 128 atoms (i) per core
NJ = N             # full j dimension on every core
MM_N = 512         # fp32 matmul max moving free dim
OPAD = 4           # W3 padded 3 -> 4 cols (aligned psum writes; pad col = 0)


def _layout(ipc, nj):
    """Column offsets of the packed constant block [H, ncols]."""
    hj = 0
    hib = hj + nj
    w2 = hib + ipc
    b2 = w2 + H
    w3 = b2 + 1
    ones = w3 + OPAD
    ncols = ones + 1
    return dict(hj=hj, hib=hib, w2=w2, b2=b2, w3=w3, ones=ones, ncols=ncols)


# TPB instructions have a single 8-byte events field: 2 sync commands max
# (walrus rejects more).  Queue-engine DMA ops handle their own sync.
_MULTIWAIT_OK = {"DMACopy", "TriggeredCopy", "Call", "ISA"}


def _legalize_waits(nc):
    """Hoist excess semaphore waits from datapath instructions onto chained
    NoOps (one wait each) so every instruction fits walrus's sync budget."""
    j = json.loads(bass_rust.module_to_json_string(nc.m))
    counter = [0]

    def fix_list(insts):
        out = []
        for inst in insts:
            si = inst.get("sync_info")
            waits = (si or {}).get("on_wait", [])
            if si and len(waits) > 1 and inst.get("opcode") not in _MULTIWAIT_OK:
                # keep zero waits on the instruction; one NoOp per wait
                for w in waits:
                    counter[0] += 1
                    out.append({
                        "debug": inst.get("debug", 0),
                        "engine": inst["engine"],
                        "ins": [],
                        "outs": [],
                        "name": f"W-hoist-{counter[0]}",
                        "opcode": "NoOp",
                        "sync_info": {"on_update": [], "on_wait": [w]},
                    })
                si["on_wait"] = []
            out.append(inst)
        return out

    def walk(o):
        if isinstance(o, dict):
            if "instructions" in o and isinstance(o["instructions"], list):
                o["instructions"] = fix_list(o["instructions"])
            for v in o.values():
                walk(v)
        elif isinstance(o, list):
            for v in o:
                walk(v)

    walk(j)
    nc.m = bass_rust.module_from_json_string(json.dumps(j))
    return counter[0]


def _build(ipc, nj, reps=1):
    """Build the per-core Bass program (SPMD: same program, per-core data).

    reps > 1 repeats the main i-loop (recomputing identical results) and is
    used only for differential timing; outputs are unchanged."""
    assert nj % MM_N == 0 and nj % H == 0
    nchunk = nj // H  # stage-3 chunks of 128 pairs
    lay = _layout(ipc, nj)

    nc = bass.Bass()
    cparam = nc.declare_dram_parameter("c", [H, lay["ncols"]], f32, isOutput=False)
    yparam = nc.declare_dram_parameter("y", [ipc, 1], f32, isOutput=True)

    with tile.TileContext(nc) as tc:
        with ExitStack() as ctx:
            consts = ctx.enter_context(tc.tile_pool(name="consts", bufs=1))
            h1p = ctx.enter_context(tc.tile_pool(name="h1p", bufs=3))
            h2p = ctx.enter_context(tc.tile_pool(name="h2p", bufs=3))
            scrp = ctx.enter_context(tc.tile_pool(name="scrp", bufs=1))
            accp = ctx.enter_context(tc.tile_pool(name="accp", bufs=1))
            # PSUM budget (8 banks): ps1 double-buffer 2x2 + ps3 2x1 + warm 1 + fin 1
            psA = ctx.enter_context(tc.tile_pool(name="psA", bufs=2, space="PSUM"))
            psB = ctx.enter_context(tc.tile_pool(name="psB", bufs=2, space="PSUM"))
            psW = ctx.enter_context(tc.tile_pool(name="psW", bufs=1, space="PSUM"))
            psF = ctx.enter_context(tc.tile_pool(name="psF", bufs=1, space="PSUM"))

            C = consts.tile([H, lay["ncols"]], f32)
            nc.sync.dma_start(out=C, in_=cparam[:, :])

            HJ = C[:, lay["hj"]:lay["hj"] + nj]
            W2 = C[:, lay["w2"]:lay["w2"] + H]
            B2 = C[:, lay["b2"]:lay["b2"] + 1]
            W3 = C[:, lay["w3"]:lay["w3"] + OPAD]
            ONES = C[:, lay["ones"]:lay["ones"] + 1]

            ACC = accp.tile([H, ipc], f32)          # [j-offset, i] partial sums
            warm = scrp.tile([H, 1], f32, tag="warm")

            # --- warmups: let ACT and PE observe the const-DMA semaphore
            # (and load the tanh table) on single-wait instructions.
            nc.scalar.activation(out=warm, in_=B2, func=Tanh)
            warm_ps = psW.tile([1, 1], f32)
            nc.tensor.matmul(warm_ps, C[:, lay["w2"]:lay["w2"] + 1],
                             C[:, lay["w2"]:lay["w2"] + 1], start=True, stop=True)

            # --- main loop: groups of G atoms; tanh1/tanh3 batched per group
            G = 8 if ipc % 8 == 0 else (4 if ipc % 4 == 0 else 1)
            for g in [g for _ in range(reps) for g in range(ipc // G)]:
                # DVE broadcast-adds HJ + hib_i into a [128, G*nj] block,
                # then ONE big ACT tanh covers the whole group.
                h1 = h1p.tile([H, G, nj], f32)
                for k in range(G):
                    i = g * G + k
                    nc.vector.tensor_scalar_add(
                        h1[:, k, :], HJ,
                        C[:, lay["hib"] + i:lay["hib"] + i + 1],
                    )
                nc.scalar.activation(out=h1[:, :, :], in_=h1[:, :, :], func=Tanh)

                ps3 = psB.tile([H, G, nchunk, OPAD], f32, tag="s3")
                for k in range(G):
                    ps1 = psA.tile([H, nj], f32)
                    for t in range(nj // MM_N):
                        nc.tensor.matmul(
                            ps1[:, t * MM_N:(t + 1) * MM_N],
                            W2,
                            h1[:, k, t * MM_N:(t + 1) * MM_N],
                            start=True, stop=True,
                        )
                    h2 = h2p.tile([H, nj], f32)
                    nc.scalar.activation(out=h2, in_=ps1, func=Tanh, bias=B2)
                    for cch in range(nchunk):
                        nc.tensor.matmul(
                            ps3[:, k, cch, :],
                            h2[:, cch * H:(cch + 1) * H],
                            W3,
                            start=True, stop=True,
                        )
                # one in-place tanh over the whole group's [128, G*32] block,
                # then DVE free-axis reduce into ACC columns
                nc.scalar.activation(out=ps3[:, :, :, :], in_=ps3[:, :, :, :],
                                     func=Tanh)
                nc.vector.tensor_reduce(
                    out=ACC[:, g * G:(g + 1) * G],
                    in_=ps3.rearrange("p g c o -> p g (c o)"),
                    axis=mybir.AxisListType.X,
                    op=mybir.AluOpType.add,
                )

            # --- reduce over the 128 j-offset partitions: out = ACC.T @ ones
            fin = psF.tile([ipc, 1], f32)
            nc.tensor.matmul(fin, ACC, ONES, start=True, stop=True)
            yout = scrp.tile([ipc, 1], f32, tag="yout")
            nc.scalar.copy(yout, fin)
            nc.sync.dma_start(out=yparam[:, :], in_=yout)

    _legalize_waits(nc)
    return nc


_NC_CACHE = {}


def _get_nc(ipc, nj):
    key = (ipc, nj)
    if key not in _NC_CACHE:
        _NC_CACHE[key] = _build(ipc, nj)
    return _NC_CACHE[key]


def _host_prep(x, W1, b1, ipc, nj):
    """Build the per-core packed const blocks. Returns list of [H,ncols] f32."""
    lay = _layout(ipc, nj)
    hi = x @ W1[:F]          # [N, H]
    hj = x @ W1[F:]          # [N, H]
    hib = hi + b1[None, :]   # fold b1
    hj_t = np.ascontiguousarray(hj[:nj].T)    # [H, nj]
    return lay, hib, hj_t


def _build_reps(reps):
    """Timing helper: the same program with the main loop repeated."""
    return _build(IPC, NJ, reps)


def make_in_maps(x, W1, b1, W2, b2, W3, b3):
    x = np.asarray(x, np.float32)
    W1 = np.asarray(W1, np.float32)
    b1 = np.asarray(b1, np.float32)
    W2 = np.asarray(W2, np.float32)
    b2 = np.asarray(b2, np.float32)
    W3 = np.asarray(W3, np.float32)

    lay, hib, hj_t = _host_prep(x, W1, b1, IPC, NJ)
    W3pad = np.zeros((H, OPAD), np.float32)
    W3pad[:, :O] = W3

    in_maps = []
    for c in range(NCORES):
        blk = np.empty((H, lay["ncols"]), np.float32)
        blk[:, lay["hj"]:lay["hj"] + NJ] = hj_t
        blk[:, lay["hib"]:lay["hib"] + IPC] = hib[c * IPC:(c + 1) * IPC].T
        blk[:, lay["w2"]:lay["w2"] + H] = W2
        blk[:, lay["b2"]] = b2
        blk[:, lay["w3"]:lay["w3"] + OPAD] = W3pad
        blk[:, lay["ones"]] = 1.0
        in_maps.append({"c": blk})
    return in_maps


def kernel(x, W1, b1, W2, b2, W3, b3):
    b3 = np.asarray(b3, np.float32)
    if np.any(b3 != 0.0):
        # Never hit for this problem (spec fills b3 with zeros); exact
        # numpy fallback keeps the kernel correct for arbitrary inputs.
        return _numpy_ref(
            np.asarray(x, np.float32), np.asarray(W1, np.float32),
            np.asarray(b1, np.float32), np.asarray(W2, np.float32),
            np.asarray(b2, np.float32), np.asarray(W3, np.float32), b3)

    in_maps = make_in_maps(x, W1, b1, W2, b2, W3, b3)
    nc = _get_nc(IPC, NJ)
    res = run_bass_kernel_spmd(nc, in_maps, list(range(NCORES)))
    out = np.concatenate(
        [res.results[c]["y"].reshape(IPC) for c in range(NCORES)]
    ).astype(np.float32)
    return out


def _numpy_ref(x, W1, b1, W2, b2, W3, b3):
    hi = x @ W1[:F]
    hj = x @ W1[F:]
    out = np.empty((N,), np.float32)
    for i in range(N):
        h = np.tanh(hi[i][None, :] + hj + b1[None, :])
        h = np.tanh(h @ W2 + b2[None, :])
        y = np.tanh(h @ W3 + b3[None, :])
        out[i] = y.sum()
    return out



# revision 5
# speedup vs baseline: 1.0235x; 1.0235x over previous
"""Trainium2 Bass kernel for nn_ConvPair (pairwise-MLP message passing).

Reference computation (N=1024 atoms, F=8 feats, H=128 hidden, O=3 out):
    hi = x @ W1[:F];  hj = x @ W1[F:]
    h  = tanh(hi[:,None,:] + hj[None,:,:] + b1)        # [N,N,H]
    h  = tanh(h @ W2 + b2)                             # [N,N,H]
    y  = tanh(h @ W3 + b3)                             # [N,N,O]
    out = y.sum(axis=(1,2))                            # [N]

Sharding: outer atom dim i split across 8 cores (128 i per core); weights and
the (host-precomputed) j-side tables replicated. No cross-core reduction.

v2 design (vs the fp32 baseline at ~790us):
  * all N^2-scale matmuls in bf16 (PE: 1 cycle/row instead of 4),
  * layer-1 tanh is split across TWO engines to beat the ACT-only floor:
      - A-path (64 i/core): DVE broadcast-add hj^T + hib_i, one batched ACT
        tanh per 8-i group (ACT ~876 ns/i).
      - D-path (64 i/core): exact identity tanh(a+b) = (ta+tb)/(1+ta*tb)
        with ta = tanh(hi_i+b1), tb = tanh(hj_j) precomputed on the HOST;
        on-device it is 4 DVE passes/i (add, mult-add, reciprocal_approx,
        mult) and costs ACT nothing (~2.6 us/i on DVE).
    The 64/64 split balances ACT (~195us) and DVE (~195us); layer-2 tanh
    stays on ACT (PSUM source). PE ~120us runs in the shadow.
  * schedule interleaves one A-group with one D-cohort per block and emits
    the D-chains one block ahead so ACT never waits on DVE.

b1 is folded into hib host-side; b2 via the ACT bias port; b3 is zeros for
this problem (asserted; a numpy fallback handles the hypothetical nonzero
case exactly).

Wait-discipline: walrus's codegen supports limited semaphore waits per
instruction, so `_legalize_waits` hoists excess waits onto chained NoOps.
"""

import json

import numpy as np
import ml_dtypes
from contextlib import ExitStack

import bass_rust
import concourse.bass as bass
import concourse.tile as tile
from concourse import mybir
from concourse.bass_utils import run_bass_kernel_spmd

f32 = mybir.dt.float32
bf16 = mybir.dt.bfloat16
Tanh = mybir.ActivationFunctionType.Tanh
ADD = mybir.AluOpType.add
MULT = mybir.AluOpType.mult

N, F, H, O = 1024, 8, 128, 3
NCORES = 8
IPC = N // NCORES  # 128 atoms (i) per core
NJ = N             # full j dimension on every core
OPAD = 4           # W3 padded 3 -> 4 cols (pad col = 0)
G = 8              # cohort size (i's per A-group / D-cohort)
NBLK = IPC // (2 * G)  # 8 blocks: each covers one A-group + one D-cohort
NCH = NJ // H      # 8 pair-chunks of 128 j per stage-3 matmul
MMN = 512          # mm1 moving chunk (PSUM bank = 512 fp32)

# bf16 const block column offsets
B_HJ, B_TB, B_W2, B_W3 = 0, NJ, 2 * NJ, 2 * NJ + H
BCOLS = 2 * NJ + H + OPAD
# f32 const block column offsets
F_HIB, F_TA, F_B2, F_ONES = 0, IPC, 2 * IPC, 2 * IPC + 1
FCOLS = 2 * IPC + 2

# TPB instructions have a single 8-byte events field: 2 sync commands max
# (walrus rejects more).  Queue-engine DMA ops handle their own sync.
_MULTIWAIT_OK = {"DMACopy", "TriggeredCopy", "Call", "ISA"}


def _legalize_waits(nc):
    """Hoist excess semaphore waits from datapath instructions onto chained
    NoOps (one wait each) so every instruction fits walrus's sync budget."""
    j = json.loads(bass_rust.module_to_json_string(nc.m))
    counter = [0]

    def fix_list(insts):
        out = []
        for inst in insts:
            si = inst.get("sync_info")
            waits = (si or {}).get("on_wait", [])
            if si and len(waits) > 1 and inst.get("opcode") not in _MULTIWAIT_OK:
                for w in waits:
                    counter[0] += 1
                    out.append({
                        "debug": inst.get("debug", 0),
                        "engine": inst["engine"],
                        "ins": [],
                        "outs": [],
                        "name": f"W-hoist-{counter[0]}",
                        "opcode": "NoOp",
                        "sync_info": {"on_update": [], "on_wait": [w]},
                    })
                si["on_wait"] = []
            out.append(inst)
        return out

    def walk(o):
        if isinstance(o, dict):
            if "instructions" in o and isinstance(o["instructions"], list):
                o["instructions"] = fix_list(o["instructions"])
            for v in o.values():
                walk(v)
        elif isinstance(o, list):
            for v in o:
                walk(v)

    walk(j)
    nc.m = bass_rust.module_from_json_string(json.dumps(j))
    return counter[0]


def _build(reps=1):
    """Build the per-core Bass program (SPMD: same program, per-core data).

    reps > 1 repeats the main loop (recomputing identical results); used
    only for differential timing."""
    nc = bass.Bass()
    cbparam = nc.declare_dram_parameter("cb", [H, BCOLS], bf16, isOutput=False)
    cfparam = nc.declare_dram_parameter("cf", [H, FCOLS], f32, isOutput=False)
    yparam = nc.declare_dram_parameter("y", [IPC, 1], f32, isOutput=True)

    with tile.TileContext(nc) as tc:
        with ExitStack() as ctx:
            cbp = ctx.enter_context(tc.tile_pool(name="cbp", bufs=1))
            cfp = ctx.enter_context(tc.tile_pool(name="cfp", bufs=1))
            h1ap = ctx.enter_context(tc.tile_pool(name="h1ap", bufs=2))
            h1dp = ctx.enter_context(tc.tile_pool(name="h1dp", bufs=2 * G))
            nump = ctx.enter_context(tc.tile_pool(name="nump", bufs=2))
            denp = ctx.enter_context(tc.tile_pool(name="denp", bufs=2))
            recp = ctx.enter_context(tc.tile_pool(name="recp", bufs=2))
            h2p = ctx.enter_context(tc.tile_pool(name="h2p", bufs=3))
            accp = ctx.enter_context(tc.tile_pool(name="accp", bufs=1))
            scrp = ctx.enter_context(tc.tile_pool(name="scrp", bufs=1))
            # PSUM budget (8 banks): ps1 2 bufs x 2 banks + ps3 2 x 1
            # + warm 1 + fin 1
            ps1p = ctx.enter_context(tc.tile_pool(name="ps1p", bufs=2,
                                                  space="PSUM"))
            ps3p = ctx.enter_context(tc.tile_pool(name="ps3p", bufs=2,
                                                  space="PSUM"))
            psW = ctx.enter_context(tc.tile_pool(name="psW", bufs=1,
                                                 space="PSUM"))
            psF = ctx.enter_context(tc.tile_pool(name="psF", bufs=1,
                                                 space="PSUM"))

            CB = cbp.tile([H, BCOLS], bf16)
            nc.sync.dma_start(out=CB, in_=cbparam[:, :])
            CF = cfp.tile([H, FCOLS], f32)
            nc.sync.dma_start(out=CF, in_=cfparam[:, :])

            HJ = CB[:, B_HJ:B_HJ + NJ]
            TB = CB[:, B_TB:B_TB + NJ]
            W2 = CB[:, B_W2:B_W2 + H]
            W3 = CB[:, B_W3:B_W3 + OPAD]
            B2 = CF[:, F_B2:F_B2 + 1]
            ONES = CF[:, F_ONES:F_ONES + 1]

            ACC = accp.tile([H, IPC], f32)          # [j-offset, i] partials
            warm = scrp.tile([H, 1], f32, tag="warm")
            warmb = scrp.tile([H, 1], bf16, tag="warmb")

            # --- warmups: every engine observes both const DMAs on
            # single-wait instructions; first Tanh loads the ACT table.
            nc.scalar.activation(out=warm, in_=B2, func=Tanh)
            nc.scalar.activation(out=warmb, in_=CB[:, 0:1], func=Tanh)
            warm_ps = psW.tile([1, 1], f32)
            nc.tensor.matmul(warm_ps, CB[:, B_W2:B_W2 + 1],
                             CB[:, B_W2:B_W2 + 1], start=True, stop=True)
            nc.tensor.matmul(warm_ps, ONES, ONES, start=True, stop=True)
            warmd = scrp.tile([H, 1], f32, tag="warmd")
            nc.vector.tensor_scalar_add(warmd, CB[:, 0:1], CF[:, 0:1])

            def emit_adds_A(b, h1a):
                """DVE: h1a[:, k, :] = HJ + hib_{8b+k} for k in 0..G."""
                for k in range(G):
                    t = G * b + k
                    nc.vector.tensor_scalar_add(
                        h1a[:, k, :], HJ, CF[:, F_HIB + t:F_HIB + t + 1])

            def emit_chain_D(b, k):
                """DVE: exact tanh(a+b) identity for atom t = 64 + 8b + k.
                Returns the finished [H, NJ] bf16 h1 tile."""
                t = NBLK * G + G * b + k
                ta = CF[:, F_TA + t:F_TA + t + 1]
                num = nump.tile([H, NJ], bf16)
                nc.vector.tensor_scalar_add(num, TB, ta)
                den = denp.tile([H, NJ], f32)
                nc.vector.tensor_scalar(out=den, in0=TB, scalar1=ta,
                                        scalar2=1.0, op0=MULT, op1=ADD)
                rec = recp.tile([H, NJ], f32)
                nc.vector.reciprocal(out=rec, in_=den)
                h1d = h1dp.tile([H, NJ], bf16)
                # final multiply runs on the otherwise-idle GPSIMD engine
                nc.gpsimd.tensor_tensor(out=h1d, in0=num, in1=rec, op=MULT)
                return h1d

            def emit_mm(h1_ap, ps3, slot):
                """PE mm1 -> ACT tanh2 -> PE mm3 x NCH for one atom."""
                ps1 = ps1p.tile([H, NJ], f32)
                for m in range(NJ // MMN):
                    nc.tensor.matmul(ps1[:, m * MMN:(m + 1) * MMN], W2,
                                     h1_ap[:, m * MMN:(m + 1) * MMN],
                                     start=True, stop=True)
                h2 = h2p.tile([H, NJ], bf16)
                nc.scalar.activation(out=h2, in_=ps1, func=Tanh, bias=B2)
                for c in range(NCH):
                    nc.tensor.matmul(ps3[:, slot, c, :],
                                     h2[:, c * H:(c + 1) * H], W3,
                                     start=True, stop=True)

            def emit_tail(ps3, col0):
                """ACT tanh3 in-place on PSUM; DVE reduce into ACC cols."""
                nc.scalar.activation(out=ps3[:, :, :, :], in_=ps3[:, :, :, :],
                                     func=Tanh)
                nc.vector.tensor_reduce(
                    out=ACC[:, col0:col0 + G],
                    in_=ps3.rearrange("p g c o -> p g (c o)"),
                    axis=mybir.AxisListType.X, op=ADD)

            for _ in range(reps):
                # prologue: A(0) adds + all D(0) chains before block 0
                h1a = h1ap.tile([H, G, NJ], bf16)
                emit_adds_A(0, h1a)
                d_tiles = [emit_chain_D(0, k) for k in range(G)]

                for b in range(NBLK):
                    # ACT: one batched tanh over the whole A-group
                    nc.scalar.activation(out=h1a[:, :, :], in_=h1a[:, :, :],
                                         func=Tanh)
                    h1a_cur = h1a
                    if b + 1 < NBLK:
                        h1a = h1ap.tile([H, G, NJ], bf16)
                        emit_adds_A(b + 1, h1a)

                    ps3a = ps3p.tile([H, G, NCH, OPAD], f32, tag="s3")
                    d_next = []
                    for k in range(G):
                        emit_mm(h1a_cur[:, k, :], ps3a, k)
                        if b + 1 < NBLK:
                            d_next.append(emit_chain_D(b + 1, k))
                    emit_tail(ps3a, G * b)

                    ps3d = ps3p.tile([H, G, NCH, OPAD], f32, tag="s3")
                    for k in range(G):
                        emit_mm(d_tiles[k], ps3d, k)
                    emit_tail(ps3d, NBLK * G + G * b)
                    d_tiles = d_next

            # --- reduce over the 128 j-offset partitions: out = ACC.T @ ones
            fin = psF.tile([IPC, 1], f32)
            nc.tensor.matmul(fin, ACC, ONES, start=True, stop=True)
            yout = scrp.tile([IPC, 1], f32, tag="yout")
            nc.scalar.copy(yout, fin)
            nc.sync.dma_start(out=yparam[:, :], in_=yout)

    _legalize_waits(nc)
    return nc


_NC_CACHE = {}


def _build_reps(reps):
    if reps not in _NC_CACHE:
        _NC_CACHE[reps] = _build(reps)
    return _NC_CACHE[reps]


def make_in_maps(x, W1, b1, W2, b2, W3, b3):
    x = np.asarray(x, np.float32)
    W1 = np.asarray(W1, np.float32)
    b1 = np.asarray(b1, np.float32)
    W2 = np.asarray(W2, np.float32)
    b2 = np.asarray(b2, np.float32)
    W3 = np.asarray(W3, np.float32)

    hi = x @ W1[:F]                       # [N, H]
    hj = x @ W1[F:]                       # [N, H]
    hib = hi + b1[None, :]                # fold b1
    hj_t = np.ascontiguousarray(hj.T)     # [H, N]
    tb_t = np.tanh(hj_t)
    W3pad = np.zeros((H, OPAD), np.float32)
    W3pad[:, :O] = W3

    cb = np.empty((H, BCOLS), ml_dtypes.bfloat16)
    cb[:, B_HJ:B_HJ + NJ] = hj_t
    cb[:, B_TB:B_TB + NJ] = tb_t
    cb[:, B_W2:B_W2 + H] = W2
    cb[:, B_W3:B_W3 + OPAD] = W3pad

    in_maps = []
    for c in range(NCORES):
        hib_c = hib[c * IPC:(c + 1) * IPC].T      # [H, IPC]
        cf = np.empty((H, FCOLS), np.float32)
        cf[:, F_HIB:F_HIB + IPC] = hib_c
        cf[:, F_TA:F_TA + IPC] = np.tanh(hib_c)
        cf[:, F_B2] = b2
        cf[:, F_ONES] = 1.0
        in_maps.append({"cb": cb, "cf": cf})
    return in_maps


def kernel(x, W1, b1, W2, b2, W3, b3):
    b3 = np.asarray(b3, np.float32)
    if np.any(b3 != 0.0):
        # Never hit for this problem (spec fills b3 with zeros); exact
        # numpy fallback keeps the kernel correct for arbitrary inputs.
        return _numpy_ref(
            np.asarray(x, np.float32), np.asarray(W1, np.float32),
            np.asarray(b1, np.float32), np.asarray(W2, np.float32),
            np.asarray(b2, np.float32), np.asarray(W3, np.float32), b3)

    in_maps = make_in_maps(x, W1, b1, W2, b2, W3, b3)
    nc = _build_reps(1)
    res = run_bass_kernel_spmd(nc, in_maps, list(range(NCORES)))
    out = np.concatenate(
        [res.results[c]["y"].reshape(IPC) for c in range(NCORES)]
    ).astype(np.float32)
    return out


def _numpy_ref(x, W1, b1, W2, b2, W3, b3):
    hi = x @ W1[:F]
    hj = x @ W1[F:]
    out = np.empty((N,), np.float32)
    for i in range(N):
        h = np.tanh(hi[i][None, :] + hj + b1[None, :])
        h = np.tanh(h @ W2 + b2[None, :])
        y = np.tanh(h @ W3 + b3[None, :])
        out[i] = y.sum()
    return out


# revision 10
# speedup vs baseline: 5.9009x; 5.7657x over previous
"""Trainium2 Bass kernel for nn_ConvPair (pairwise-MLP message passing).

Reference computation (N=1024 atoms, F=8 feats, H=128 hidden, O=3 out):
    hi = x @ W1[:F];  hj = x @ W1[F:]
    h  = tanh(hi[:,None,:] + hj[None,:,:] + b1)        # [N,N,H]
    h  = tanh(h @ W2 + b2)                             # [N,N,H]
    y  = tanh(h @ W3 + b3)                             # [N,N,O]
    out = y.sum(axis=(1,2))                            # [N]

Sharding: outer atom dim i split across 8 cores (128 i per core); weights and
the (host-precomputed) j-side tables replicated. No cross-core reduction.

v2 design (vs the fp32 baseline at ~790us):
  * all N^2-scale matmuls in bf16 (PE: 1 cycle/row instead of 4),
  * layer-1 tanh is split across TWO engines to beat the ACT-only floor:
      - A-path (64 i/core): DVE broadcast-add hj^T + hib_i, one batched ACT
        tanh per 8-i group (ACT ~876 ns/i).
      - D-path (64 i/core): exact identity tanh(a+b) = (ta+tb)/(1+ta*tb)
        with ta = tanh(hi_i+b1), tb = tanh(hj_j) precomputed on the HOST;
        on-device it is 4 DVE passes/i (add, mult-add, reciprocal_approx,
        mult) and costs ACT nothing (~2.6 us/i on DVE).
    The 64/64 split balances ACT (~195us) and DVE (~195us); layer-2 tanh
    stays on ACT (PSUM source). PE ~120us runs in the shadow.
  * schedule interleaves one A-group with one D-cohort per block and emits
    the D-chains one block ahead so ACT never waits on DVE.

b1 is folded into hib host-side; b2 via the ACT bias port; b3 is zeros for
this problem (asserted; a numpy fallback handles the hypothetical nonzero
case exactly).

Wait-discipline: walrus's codegen supports limited semaphore waits per
instruction, so `_legalize_waits` hoists excess waits onto chained NoOps.
"""

import json

import numpy as np
import ml_dtypes
from contextlib import ExitStack

import bass_rust
import concourse.bass as bass
import concourse.tile as tile
from concourse import mybir
from concourse.bass_utils import run_bass_kernel_spmd

f32 = mybir.dt.float32
bf16 = mybir.dt.bfloat16
Tanh = mybir.ActivationFunctionType.Tanh
ADD = mybir.AluOpType.add
MULT = mybir.AluOpType.mult

N, F, H, O = 1024, 8, 128, 3
NCORES = 8
IPC = N // NCORES  # 128 atoms (i) per core
NJ = N             # full j dimension on every core
OPAD = 4           # W3 padded 3 -> 4 cols (pad col = 0)
G = 8              # cohort size (i's per A-group / D-cohort)
NBLK = IPC // (2 * G)  # 8 blocks: each covers one A-group + one D-cohort
NCH = NJ // H      # 8 pair-chunks of 128 j per stage-3 matmul
MMN = 512          # mm1 moving chunk (PSUM bank = 512 fp32)

# bf16 const block column offsets
B_HJ, B_W2, B_W3 = 0, NJ, NJ + H
BCOLS = NJ + H + OPAD
# f32 const block column offsets
F_HIB, F_B2, F_ONES = 0, IPC, IPC + 1
FCOLS = IPC + 2

# TPB instructions have a single 8-byte events field: 2 sync commands max
# (walrus rejects more).  Queue-engine DMA ops handle their own sync.
_MULTIWAIT_OK = {"DMACopy", "TriggeredCopy", "Call", "ISA"}


def _legalize_waits(nc):
    """Hoist excess semaphore waits from datapath instructions onto chained
    NoOps (one wait each) so every instruction fits walrus's sync budget."""
    j = json.loads(bass_rust.module_to_json_string(nc.m))
    counter = [0]

    def fix_list(insts):
        out = []
        for inst in insts:
            si = inst.get("sync_info")
            waits = (si or {}).get("on_wait", [])
            if si and len(waits) > 1 and inst.get("opcode") not in _MULTIWAIT_OK:
                for w in waits:
                    counter[0] += 1
                    out.append({
                        "debug": inst.get("debug", 0),
                        "engine": inst["engine"],
                        "ins": [],
                        "outs": [],
                        "name": f"W-hoist-{counter[0]}",
                        "opcode": "NoOp",
                        "sync_info": {"on_update": [], "on_wait": [w]},
                    })
                si["on_wait"] = []
            out.append(inst)
        return out

    def walk(o):
        if isinstance(o, dict):
            if "instructions" in o and isinstance(o["instructions"], list):
                o["instructions"] = fix_list(o["instructions"])
            for v in o.values():
                walk(v)
        elif isinstance(o, list):
            for v in o:
                walk(v)

    walk(j)
    nc.m = bass_rust.module_from_json_string(json.dumps(j))
    return counter[0]


def _build(reps=1):
    """Build the per-core Bass program (SPMD: same program, per-core data).

    reps > 1 repeats the main loop (recomputing identical results); used
    only for differential timing."""
    nc = bass.Bass()
    cbparam = nc.declare_dram_parameter("cb", [H, BCOLS], bf16, isOutput=False)
    cfparam = nc.declare_dram_parameter("cf", [H, FCOLS], f32, isOutput=False)
    yparam = nc.declare_dram_parameter("y", [IPC, 1], f32, isOutput=True)

    with tile.TileContext(nc) as tc:
        with ExitStack() as ctx:
            cbp = ctx.enter_context(tc.tile_pool(name="cbp", bufs=1))
            cfp = ctx.enter_context(tc.tile_pool(name="cfp", bufs=1))
            h1p = ctx.enter_context(tc.tile_pool(name="h1p", bufs=3))
            h2p = ctx.enter_context(tc.tile_pool(name="h2p", bufs=3))
            accp = ctx.enter_context(tc.tile_pool(name="accp", bufs=1))
            scrp = ctx.enter_context(tc.tile_pool(name="scrp", bufs=1))
            # PSUM budget (8 banks): ps1 2 bufs x 2 banks + ps3 3 x 1
            # + warm/fin 1
            ps1p = ctx.enter_context(tc.tile_pool(name="ps1p", bufs=2,
                                                  space="PSUM"))
            ps3p = ctx.enter_context(tc.tile_pool(name="ps3p", bufs=3,
                                                  space="PSUM"))
            psW = ctx.enter_context(tc.tile_pool(name="psW", bufs=1,
                                                 space="PSUM"))

            CB = cbp.tile([H, BCOLS], bf16)
            nc.sync.dma_start(out=CB, in_=cbparam[:, :])
            CF = cfp.tile([H, FCOLS], f32)
            nc.sync.dma_start(out=CF, in_=cfparam[:, :])

            HJ = CB[:, B_HJ:B_HJ + NJ]
            W2 = CB[:, B_W2:B_W2 + H]
            W3 = CB[:, B_W3:B_W3 + OPAD]
            B2 = CF[:, F_B2:F_B2 + 1]
            ONES = CF[:, F_ONES:F_ONES + 1]

            ACC = accp.tile([H, IPC], f32)          # [j-offset, i] partials
            warm = scrp.tile([H, 1], f32, tag="warm")
            warmb = scrp.tile([H, 1], bf16, tag="warmb")

            # --- warmups: every engine observes both const DMAs on
            # single-wait instructions; first Tanh loads the ACT table.
            nc.scalar.activation(out=warm, in_=B2, func=Tanh)
            nc.scalar.activation(out=warmb, in_=CB[:, 0:1], func=Tanh)
            warm_ps = psW.tile([IPC, 1], f32, tag="wf")
            nc.tensor.matmul(warm_ps[0:1, 0:1], CB[:, B_W2:B_W2 + 1],
                             CB[:, B_W2:B_W2 + 1], start=True, stop=True)
            nc.tensor.matmul(warm_ps[0:1, 0:1], ONES, ONES,
                             start=True, stop=True)
            warmd = scrp.tile([H, 1], f32, tag="warmd")
            nc.vector.tensor_scalar_add(warmd, CB[:, 0:1], CF[:, 0:1])

            def emit_tanh1(t):
                """ACT: h1 = tanh(HJ + hib_t) via the free bias port."""
                h1 = h1p.tile([H, NJ], bf16)
                nc.scalar.activation(out=h1, in_=HJ, func=Tanh,
                                     bias=CF[:, F_HIB + t:F_HIB + t + 1])
                return h1

            def emit_mm(h1, ps3, slot):
                """PE mm1 -> ACT tanh2 -> PE mm3 x NCH for one atom."""
                ps1 = ps1p.tile([H, NJ], f32)
                for m in range(NJ // MMN):
                    nc.tensor.matmul(ps1[:, m * MMN:(m + 1) * MMN], W2,
                                     h1[:, m * MMN:(m + 1) * MMN],
                                     start=True, stop=True)
                h2 = h2p.tile([H, NJ], bf16)
                nc.scalar.activation(out=h2, in_=ps1, func=Tanh, bias=B2)
                for c in range(NCH):
                    nc.tensor.matmul(ps3[:, slot, c, :],
                                     h2[:, c * H:(c + 1) * H], W3,
                                     start=True, stop=True)

            def emit_tail(ps3, col0):
                """ACT tanh3 in-place on PSUM; DVE reduce into ACC cols."""
                nc.scalar.activation(out=ps3[:, :, :, :], in_=ps3[:, :, :, :],
                                     func=Tanh)
                nc.vector.tensor_reduce(
                    out=ACC[:, col0:col0 + G],
                    in_=ps3.rearrange("p g c o -> p g (c o)"),
                    axis=mybir.AxisListType.X, op=ADD)

            for _ in range(reps):
                # software pipeline: ACT produces tanh1(i+1) while PE works
                # on mm1(i), so ACT alternates tanh1/tanh2 without stalls.
                h1_next = emit_tanh1(0)
                for g in range(IPC // G):
                    ps3 = ps3p.tile([H, G, NCH, OPAD], f32, tag="s3")
                    for k in range(G):
                        t = G * g + k
                        h1 = h1_next
                        if t + 1 < IPC:
                            h1_next = emit_tanh1(t + 1)
                        emit_mm(h1, ps3, k)
                    emit_tail(ps3, G * g)

            # --- reduce over the 128 j-offset partitions: out = ACC.T @ ones
            nc.tensor.matmul(warm_ps, ACC, ONES, start=True, stop=True)
            yout = scrp.tile([IPC, 1], f32, tag="yout")
            nc.scalar.copy(yout, warm_ps)
            nc.sync.dma_start(out=yparam[:, :], in_=yout)

    _legalize_waits(nc)
    return nc


_NC_CACHE = {}


def _build_reps(reps):
    if reps not in _NC_CACHE:
        _NC_CACHE[reps] = _build(reps)
    return _NC_CACHE[reps]


def make_in_maps(x, W1, b1, W2, b2, W3, b3):
    x = np.asarray(x, np.float32)
    W1 = np.asarray(W1, np.float32)
    b1 = np.asarray(b1, np.float32)
    W2 = np.asarray(W2, np.float32)
    b2 = np.asarray(b2, np.float32)
    W3 = np.asarray(W3, np.float32)

    hi = x @ W1[:F]                       # [N, H]
    hj = x @ W1[F:]                       # [N, H]
    hib = hi + b1[None, :]                # fold b1
    hj_t = np.ascontiguousarray(hj.T)     # [H, N]
    W3pad = np.zeros((H, OPAD), np.float32)
    W3pad[:, :O] = W3

    cb = np.empty((H, BCOLS), ml_dtypes.bfloat16)
    cb[:, B_HJ:B_HJ + NJ] = hj_t
    cb[:, B_W2:B_W2 + H] = W2
    cb[:, B_W3:B_W3 + OPAD] = W3pad

    in_maps = []
    for c in range(NCORES):
        hib_c = hib[c * IPC:(c + 1) * IPC].T      # [H, IPC]
        cf = np.empty((H, FCOLS), np.float32)
        cf[:, F_HIB:F_HIB + IPC] = hib_c
        cf[:, F_B2] = b2
        cf[:, F_ONES] = 1.0
        in_maps.append({"cb": cb, "cf": cf})
    return in_maps


def kernel(x, W1, b1, W2, b2, W3, b3):
    b3 = np.asarray(b3, np.float32)
    if np.any(b3 != 0.0):
        # Never hit for this problem (spec fills b3 with zeros); exact
        # numpy fallback keeps the kernel correct for arbitrary inputs.
        return _numpy_ref(
            np.asarray(x, np.float32), np.asarray(W1, np.float32),
            np.asarray(b1, np.float32), np.asarray(W2, np.float32),
            np.asarray(b2, np.float32), np.asarray(W3, np.float32), b3)

    in_maps = make_in_maps(x, W1, b1, W2, b2, W3, b3)
    nc = _build_reps(1)
    res = run_bass_kernel_spmd(nc, in_maps, list(range(NCORES)))
    out = np.concatenate(
        [res.results[c]["y"].reshape(IPC) for c in range(NCORES)]
    ).astype(np.float32)
    return out


def _numpy_ref(x, W1, b1, W2, b2, W3, b3):
    hi = x @ W1[:F]
    hj = x @ W1[F:]
    out = np.empty((N,), np.float32)
    for i in range(N):
        h = np.tanh(hi[i][None, :] + hj + b1[None, :])
        h = np.tanh(h @ W2 + b2[None, :])
        y = np.tanh(h @ W3 + b3[None, :])
        out[i] = y.sum()
    return out
